# revision 8
# baseline (speedup 1.0000x reference)
"""Trainium2 Bass kernel for nn_LocalRNN (local GRU, chunked scan).

Problem: B=32, S=2048, I=H=256, ksize=16. Each ksize-chunk runs a GRU from
h0=0, so the 32*128=4096 chunks are independent length-16 GRU chains.

Sharding: data-parallel over chunks — core c gets batch rows [4c:4c+4],
i.e. 512 chains. Weights replicated.

Per-core layout ("transposed"): gate/hidden dim on partitions, chain (seq)
index on the free dim, all NS=512 chains in one matmul (N=512 = one PSUM
bank of fp32). Per step t, for each gate-half m (2 halves of 128):

  psum[gate_m, seqs] = W_ih_m @ x_t^T (+ W_hh_m @ h_{t-1}^T)   (PE, fp16)
  r = sigmoid(psum_r + b_r)                    (ScalarE bias port)
  z = sigmoid(psum_z + b_z)
  n = tanh((psum_in + b_in) + r*(psum_hn + b_hn))  (DVE stt ops + ScalarE)
  h = n + z*(h_prev - n)                       (DVE d/e/h chain)

The 8 PSUM banks hold r0,r1,z0,z1,in0,in1,hn0,hn1 single-buffered; step
t+1's x-side matmuls are emitted right after step t's h-side matmuls so the
PE pipelines across the elementwise chain (x-side needs no h). h-side
matmuls are ordered so k0-consumers lead (h half 0 lands ~1us before half 1).

Matmul operands and elementwise SBUF tensors are fp16 (values are O(1));
PSUM accumulation is fp32. Weight DMAs ride the Sync queue while x tiles
ride the GpSimd queue so the startup transfers overlap. Host pre-transposes
x / weights into DMA-friendly contiguous blocks and inverts the output
layout at the end.
"""

import sys

for _p in ("/opt/trn_rl_repo", "/root/.axon_site"):
    if _p not in sys.path:
        sys.path.insert(0, _p)

import ml_dtypes  # noqa: F401
import numpy as np

import concourse.bass as bass  # noqa: F401
import concourse.tile as tile
from concourse import bacc, mybir
from concourse.bass_utils import run_bass_kernel_spmd

# Problem constants (hardcoded per harness contract).
B, S, I, H = 32, 2048, 256, 256
KSIZE = 16
NCORES = 8
ROWS_PER_CORE = B // NCORES            # 4 batch rows per core
CHUNKS_PER_ROW = S // KSIZE            # 128
NS = ROWS_PER_CORE * CHUNKS_PER_ROW    # 512 chains per core
KT = 2                                 # contraction tiles (I/128 = H/128 = 2)

F32 = mybir.dt.float32
F16 = mybir.dt.float16
AF = mybir.ActivationFunctionType
OP = mybir.AluOpType

MM_DT = F16
NP_MM_DT = np.float16

# Gate column bases in the 3H weight layout.
GCOL = {"r": 0, "z": 256, "n": 512}


def build_nc():
    nc = bacc.Bacc("TRN2", target_bir_lowering=False, debug=False)

    # Inputs (host pre-transposed, contiguous per-DMA blocks).
    # xt[t, p, k, s] = x_shard[seq=s, t, i=k*128+p]
    xt_d = nc.dram_tensor("xt", [KSIZE, 128, KT, NS], MM_DT, kind="ExternalInput")
    # wih split so the first (x-side r/z) matmuls can start sooner.
    wih_rz_d = nc.dram_tensor("wih_rz", [128, KT, 512], MM_DT, kind="ExternalInput")
    wih_n_d = nc.dram_tensor("wih_n", [128, KT, 256], MM_DT, kind="ExternalInput")
    whh_d = nc.dram_tensor("whh_t", [128, KT, 3 * H], MM_DT, kind="ExternalInput")
    # brz[p, mi] = (b_ih+b_hh)[mi*128+p] for mi in r0,r1,z0,z1
    brz_d = nc.dram_tensor("brz", [128, 4], F32, kind="ExternalInput")
    # bhn[p, m] = b_hh[2H + m*128 + p]; bin[p, m] = b_ih[2H + m*128 + p]
    bhn_d = nc.dram_tensor("bhn", [128, 2], F32, kind="ExternalInput")
    bin_d = nc.dram_tensor("bin", [128, 2], F32, kind="ExternalInput")
    # out[t, m, p, s] = h_t[seq=s, hdim=m*128+p]
    out_d = nc.dram_tensor("out", [KSIZE, 2, 128, NS], MM_DT, kind="ExternalOutput")

    with tile.TileContext(nc) as tc:
        with (
            tc.tile_pool(name="consts", bufs=1) as consts,
            tc.tile_pool(name="xp", bufs=KSIZE) as xp,
            tc.tile_pool(name="ps", bufs=1, space="PSUM") as ps,
            tc.tile_pool(name="work", bufs=2) as work,
            tc.tile_pool(name="hp", bufs=3) as hp,
        ):
            # --- Input DMAs. Weights/biases on the Sync queue, x tiles on
            # the GpSimd queue: the transfers overlap at startup.
            wih = consts.tile([128, KT, 3 * H], MM_DT)
            nc.sync.dma_start(wih[:, :, 0:512], wih_rz_d.ap())
            xs = []
            for t in range(KSIZE):
                xs.append(
                    xp.tile([128, KT, NS], MM_DT, tag="x", name=f"xs{t}")
                )
                nc.gpsimd.dma_start(xs[t][:], xt_d.ap()[t])
            nc.sync.dma_start(wih[:, :, 512:768], wih_n_d.ap())
            brz = consts.tile([128, 4], F32)
            nc.sync.dma_start(brz[:], brz_d.ap())
            bhn = consts.tile([128, 2], F32)
            nc.sync.dma_start(bhn[:], bhn_d.ap())
            bin_ = consts.tile([128, 2], F32)
            nc.sync.dma_start(bin_[:], bin_d.ap())
            whh = consts.tile([128, KT, 3 * H], MM_DT)
            nc.sync.dma_start(whh[:], whh_d.ap())

            def new_banks():
                return {
                    q: [
                        ps.tile(
                            [128, NS], F32, tag=f"{q}{m}", name=f"bank_{q}{m}"
                        )
                        for m in range(2)
                    ]
                    for q in ("r", "z", "in", "hn")
                }

            def emit_x_mms(t, banks):
                """x-side matmuls for step t (emitted during step t-1).
                r/z first (their banks free earliest), in last."""
                stop_rz = t == 0  # no h-side at t=0: x k1 closes the group
                for q, m, stop in (
                    ("r", 0, stop_rz), ("r", 1, stop_rz),
                    ("z", 0, stop_rz), ("z", 1, stop_rz),
                    ("in", 0, True), ("in", 1, True),
                ):
                    col = slice(GCOL[q if q != "in" else "n"] + m * 128,
                                GCOL[q if q != "in" else "n"] + (m + 1) * 128)
                    for k in range(KT):
                        nc.tensor.matmul(
                            banks[q][m][:], wih[:, k, col], xs[t][:, k, :],
                            start=(k == 0), stop=(stop and k == KT - 1),
                        )

            def emit_h_mms(t, banks, hprev):
                """h-side matmuls for step t. k0-consumers first (h half 0 is
                ready ~1us before half 1); r gate leads the EW chain."""
                order = [
                    ("r", 0, 0), ("r", 1, 0), ("r", 0, 1), ("r", 1, 1),
                    ("hn", 0, 0), ("hn", 1, 0), ("hn", 0, 1), ("hn", 1, 1),
                    ("z", 0, 0), ("z", 0, 1), ("z", 1, 0), ("z", 1, 1),
                ]
                for q, m, k in order:
                    col = slice(GCOL[q if q != "hn" else "n"] + m * 128,
                                GCOL[q if q != "hn" else "n"] + (m + 1) * 128)
                    nc.tensor.matmul(
                        banks[q][m][:], whh[:, k, col], hprev[:, k, :],
                        start=(q == "hn" and k == 0), stop=(k == KT - 1),
                    )

            cur = new_banks()
            emit_x_mms(0, cur)
            hprev = None
            for t in range(KSIZE):
                if t > 0:
                    emit_h_mms(t, cur, hprev[:])
                if t < KSIZE - 1:
                    nxt = new_banks()
                    emit_x_mms(t + 1, nxt)
                else:
                    nxt = None

                # --- Elementwise chain for step t.
                r_t = work.tile([128, 2, NS], MM_DT, tag="rg")
                z_t = work.tile([128, 2, NS], MM_DT, tag="zg")
                tmp = work.tile([128, 2, NS], MM_DT, tag="tmp")
                pren = work.tile([128, 2, NS], MM_DT, tag="pren")
                n_t = work.tile([128, 2, NS], MM_DT, tag="n")
                d_t = work.tile([128, 2, NS], MM_DT, tag="d")
                e_t = work.tile([128, 2, NS], MM_DT, tag="e")
                hnew = hp.tile([128, 2, NS], MM_DT, tag="h")

                # ScalarE queue: r0, r1, z0, tanh0, z1, tanh1
                nc.scalar.activation(
                    r_t[:, 0, :], cur["r"][0][:], AF.Sigmoid, bias=brz[:, 0:1]
                )
                nc.scalar.activation(
                    r_t[:, 1, :], cur["r"][1][:], AF.Sigmoid, bias=brz[:, 1:2]
                )
                nc.scalar.activation(
                    z_t[:, 0, :], cur["z"][0][:], AF.Sigmoid, bias=brz[:, 2:3]
                )

                # DVE queue: tmp0, pren0, tmp1, pren1, then d/e/h per half.
                for m in range(2):
                    if t == 0:
                        nc.vector.tensor_scalar_mul(
                            tmp[:, m, :], r_t[:, m, :], bhn[:, m : m + 1]
                        )
                    else:
                        nc.vector.scalar_tensor_tensor(
                            tmp[:, m, :], cur["hn"][m][:], bhn[:, m : m + 1],
                            r_t[:, m, :], op0=OP.add, op1=OP.mult,
                        )
                    nc.vector.scalar_tensor_tensor(
                        pren[:, m, :], cur["in"][m][:], bin_[:, m : m + 1],
                        tmp[:, m, :], op0=OP.add, op1=OP.add,
                    )
                    if m == 0:
                        nc.scalar.activation(
                            n_t[:, 0, :], pren[:, 0, :], AF.Tanh
                        )
                        nc.scalar.activation(
                            z_t[:, 1, :], cur["z"][1][:], AF.Sigmoid,
                            bias=brz[:, 3:4],
                        )
                    else:
                        nc.scalar.activation(
                            n_t[:, 1, :], pren[:, 1, :], AF.Tanh
                        )

                # DVE tail per half: d = h_prev - n; e = z*d; h = n + e.
                # (t=0: e = z*n; h = n - e.)  Final step runs quarter-sized
                # pieces so the last output DMA starts sooner.
                spl = (
                    [(0, NS // 2), (NS // 2, NS)] if t == KSIZE - 1
                    else [(0, NS)]
                )
                for m in range(2):
                    for s0, s1 in spl:
                        if t == 0:
                            nc.vector.tensor_tensor(
                                e_t[:, m, s0:s1], z_t[:, m, s0:s1],
                                n_t[:, m, s0:s1], op=OP.mult,
                            )
                            nc.vector.tensor_tensor(
                                hnew[:, m, s0:s1], n_t[:, m, s0:s1],
                                e_t[:, m, s0:s1], op=OP.subtract,
                            )
                        else:
                            nc.vector.tensor_tensor(
                                d_t[:, m, s0:s1], hprev[:, m, s0:s1],
                                n_t[:, m, s0:s1], op=OP.subtract,
                            )
                            nc.vector.tensor_tensor(
                                e_t[:, m, s0:s1], z_t[:, m, s0:s1],
                                d_t[:, m, s0:s1], op=OP.mult,
                            )
                            nc.vector.tensor_tensor(
                                hnew[:, m, s0:s1], n_t[:, m, s0:s1],
                                e_t[:, m, s0:s1], op=OP.add,
                            )
                        nc.gpsimd.dma_start(
                            out_d.ap()[t, m][:, s0:s1], hnew[:, m, s0:s1]
                        )

                hprev = hnew
                cur = nxt

    nc.compile()
    return nc


_NC_CACHE = None


def _get_nc():
    global _NC_CACHE
    if _NC_CACHE is None:
        _NC_CACHE = build_nc()
    return _NC_CACHE


def _prep_shared(W_ih, W_hh, b_ih, b_hh):
    wih_t = np.ascontiguousarray(
        W_ih.T.reshape(KT, 128, 3 * H).transpose(1, 0, 2)
    ).astype(NP_MM_DT)
    whh_t = np.ascontiguousarray(
        W_hh.T.reshape(KT, 128, 3 * H).transpose(1, 0, 2)
    ).astype(NP_MM_DT)
    wih_rz = np.ascontiguousarray(wih_t[:, :, 0:512])
    wih_n = np.ascontiguousarray(wih_t[:, :, 512:768])
    bsum = (b_ih + b_hh).astype(np.float32)
    brz = np.ascontiguousarray(bsum[: 2 * H].reshape(4, 128).T)
    bhn = np.ascontiguousarray(b_hh[2 * H :].reshape(2, 128).T)
    bin_ = np.ascontiguousarray(b_ih[2 * H :].reshape(2, 128).T)
    return wih_rz, wih_n, whh_t, brz, bhn, bin_


def _prep_core_inputs(x, shared, core):
    wih_rz, wih_n, whh_t, brz, bhn, bin_ = shared
    xc = x[core * ROWS_PER_CORE : (core + 1) * ROWS_PER_CORE]  # [4, S, I]
    xc = xc.reshape(NS, KSIZE, I)
    # xt[t, p, k, s] = xc[s, t, k*128+p]
    xt = np.ascontiguousarray(
        xc.reshape(NS, KSIZE, KT, 128).transpose(1, 3, 2, 0)
    ).astype(NP_MM_DT)
    return {
        "xt": xt,
        "wih_rz": wih_rz,
        "wih_n": wih_n,
        "whh_t": whh_t,
        "brz": brz,
        "bhn": bhn,
        "bin": bin_,
    }


def kernel(x, W_ih, W_hh, b_ih, b_hh, ksize):
    x = np.asarray(x, dtype=np.float32)
    W_ih = np.asarray(W_ih, dtype=np.float32)
    W_hh = np.asarray(W_hh, dtype=np.float32)
    b_ih = np.asarray(b_ih, dtype=np.float32)
    b_hh = np.asarray(b_hh, dtype=np.float32)
    assert int(ksize) == KSIZE and x.shape == (B, S, I)

    shared = _prep_shared(W_ih, W_hh, b_ih, b_hh)
    in_maps = [_prep_core_inputs(x, shared, c) for c in range(NCORES)]
    nc = _get_nc()
    res = run_bass_kernel_spmd(nc, in_maps, core_ids=list(range(NCORES)))

    out = np.empty((B, S, H), dtype=np.float32)
    for c in range(NCORES):
        oc = np.asarray(res.results[c]["out"]).astype(np.float32)  # [t,m,p,s]
        # h[seq=s, t, hdim=m*128+p]
        hc = oc.transpose(3, 0, 1, 2).reshape(NS, KSIZE, H)
        out[c * ROWS_PER_CORE : (c + 1) * ROWS_PER_CORE] = hc.reshape(
            ROWS_PER_CORE, S, H
        )
    return out


# revision 11
# speedup vs baseline: 1.0171x; 1.0171x over previous
"""Trainium2 Bass kernel for nn_LocalRNN (local GRU, chunked scan).

Problem: B=32, S=2048, I=H=256, ksize=16. Each ksize-chunk runs a GRU from
h0=0, so the 32*128=4096 chunks are independent length-16 GRU chains.

Sharding: data-parallel over chunks — core c gets batch rows [4c:4c+4],
i.e. 512 chains. Weights replicated.

Per-core layout ("transposed"): gate/hidden dim on partitions, chain (seq)
index on the free dim, all NS=512 chains in one matmul (N=512 = one PSUM
bank of fp32). Per step t, for each gate-half m (2 halves of 128):

  psum[gate_m, seqs] = W_ih_m @ x_t^T (+ W_hh_m @ h_{t-1}^T)   (PE, fp16)
  r = sigmoid(psum_r + b_r)                    (ScalarE bias port)
  z = sigmoid(psum_z + b_z)
  n = tanh((psum_in + b_in) + r*(psum_hn + b_hn))  (DVE stt ops + ScalarE)
  h = n + z*(h_prev - n)                       (DVE d/e/h chain)

The 8 PSUM banks hold r0,r1,z0,z1,in0,in1,hn0,hn1 single-buffered; step
t+1's x-side matmuls are emitted right after step t's h-side matmuls so the
PE pipelines across the elementwise chain (x-side needs no h). h-side
matmuls are ordered so k0-consumers lead (h half 0 lands ~1us before half 1).

Matmul operands and elementwise SBUF tensors are fp16 (values are O(1));
PSUM accumulation is fp32. Weight DMAs ride the Sync queue while x tiles
ride the GpSimd queue so the startup transfers overlap. Host pre-transposes
x / weights into DMA-friendly contiguous blocks and inverts the output
layout at the end.
"""

import sys

for _p in ("/opt/trn_rl_repo", "/root/.axon_site"):
    if _p not in sys.path:
        sys.path.insert(0, _p)

import ml_dtypes  # noqa: F401
import numpy as np

import concourse.bass as bass  # noqa: F401
import concourse.tile as tile
from concourse import bacc, mybir
from concourse.bass_utils import run_bass_kernel_spmd

# Problem constants (hardcoded per harness contract).
B, S, I, H = 32, 2048, 256, 256
KSIZE = 16
NCORES = 8
ROWS_PER_CORE = B // NCORES            # 4 batch rows per core
CHUNKS_PER_ROW = S // KSIZE            # 128
NS = ROWS_PER_CORE * CHUNKS_PER_ROW    # 512 chains per core
KT = 2                                 # contraction tiles (I/128 = H/128 = 2)

F32 = mybir.dt.float32
F16 = mybir.dt.float16
AF = mybir.ActivationFunctionType
OP = mybir.AluOpType

MM_DT = F16
NP_MM_DT = np.float16

# Gate column bases in the 3H weight layout.
GCOL = {"r": 0, "z": 256, "n": 512}


def build_nc():
    nc = bacc.Bacc("TRN2", target_bir_lowering=False, debug=False)

    # Inputs (host pre-transposed, contiguous per-DMA blocks).
    # xt[t, p, k, s] = x_shard[seq=s, t, i=k*128+p]
    xt_d = nc.dram_tensor("xt", [KSIZE, 128, KT, NS], MM_DT, kind="ExternalInput")
    # wih split so the first (x-side r/z) matmuls can start sooner.
    wih_rz_d = nc.dram_tensor("wih_rz", [128, KT, 512], MM_DT, kind="ExternalInput")
    wih_n_d = nc.dram_tensor("wih_n", [128, KT, 256], MM_DT, kind="ExternalInput")
    whh_d = nc.dram_tensor("whh_t", [128, KT, 3 * H], MM_DT, kind="ExternalInput")
    # brz[p, mi] = (b_ih+b_hh)[mi*128+p] for mi in r0,r1,z0,z1
    brz_d = nc.dram_tensor("brz", [128, 4], F32, kind="ExternalInput")
    # bhn[p, m] = b_hh[2H + m*128 + p]; bin[p, m] = b_ih[2H + m*128 + p]
    bhn_d = nc.dram_tensor("bhn", [128, 2], F32, kind="ExternalInput")
    bin_d = nc.dram_tensor("bin", [128, 2], F32, kind="ExternalInput")
    # out[t, m, p, s] = h_t[seq=s, hdim=m*128+p]
    out_d = nc.dram_tensor("out", [KSIZE, 2, 128, NS], MM_DT, kind="ExternalOutput")

    with tile.TileContext(nc) as tc:
        with (
            tc.tile_pool(name="consts", bufs=1) as consts,
            tc.tile_pool(name="xp", bufs=KSIZE) as xp,
            tc.tile_pool(name="ps", bufs=1, space="PSUM") as ps,
            tc.tile_pool(name="work", bufs=2) as work,
            tc.tile_pool(name="hp", bufs=3) as hp,
        ):
            # --- Input DMAs, ordered by first use: r/z input weights, the
            # first x tiles, then the rest.
            wih = consts.tile([128, KT, 3 * H], MM_DT)
            nc.sync.dma_start(wih[:, :, 0:512], wih_rz_d.ap())
            xs = []
            for t in range(KSIZE):
                xs.append(
                    xp.tile([128, KT, NS], MM_DT, tag="x", name=f"xs{t}")
                )
            nc.sync.dma_start(xs[0][:], xt_d.ap()[0])
            nc.sync.dma_start(wih[:, :, 512:768], wih_n_d.ap())
            nc.sync.dma_start(xs[1][:], xt_d.ap()[1])
            brz = consts.tile([128, 4], F32)
            nc.sync.dma_start(brz[:], brz_d.ap())
            bhn = consts.tile([128, 2], F32)
            nc.sync.dma_start(bhn[:], bhn_d.ap())
            bin_ = consts.tile([128, 2], F32)
            nc.sync.dma_start(bin_[:], bin_d.ap())
            whh = consts.tile([128, KT, 3 * H], MM_DT)
            nc.sync.dma_start(whh[:], whh_d.ap())
            for t in range(2, KSIZE):
                nc.sync.dma_start(xs[t][:], xt_d.ap()[t])

            def new_banks():
                return {
                    q: [
                        ps.tile(
                            [128, NS], F32, tag=f"{q}{m}", name=f"bank_{q}{m}"
                        )
                        for m in range(2)
                    ]
                    for q in ("r", "z", "in", "hn")
                }

            def emit_x_mms(t, banks):
                """x-side matmuls for step t (emitted during step t-1).
                r/z first (their banks free earliest), in last."""
                stop_rz = t == 0  # no h-side at t=0: x k1 closes the group
                for q, m, stop in (
                    ("r", 0, stop_rz), ("r", 1, stop_rz),
                    ("in", 0, True), ("in", 1, True),
                    ("z", 0, stop_rz), ("z", 1, stop_rz),
                ):
                    col = slice(GCOL[q if q != "in" else "n"] + m * 128,
                                GCOL[q if q != "in" else "n"] + (m + 1) * 128)
                    for k in range(KT):
                        nc.tensor.matmul(
                            banks[q][m][:], wih[:, k, col], xs[t][:, k, :],
                            start=(k == 0), stop=(stop and k == KT - 1),
                        )

            def emit_h_mms(t, banks, hprev):
                """h-side matmuls for step t. k0-consumers first (h half 0 is
                ready ~1us before half 1); r gate leads the EW chain."""
                order = [
                    ("r", 0, 0), ("r", 1, 0), ("r", 0, 1), ("r", 1, 1),
                    ("hn", 0, 0), ("hn", 1, 0), ("hn", 0, 1), ("hn", 1, 1),
                    ("z", 0, 0), ("z", 0, 1), ("z", 1, 0), ("z", 1, 1),
                ]
                for q, m, k in order:
                    col = slice(GCOL[q if q != "hn" else "n"] + m * 128,
                                GCOL[q if q != "hn" else "n"] + (m + 1) * 128)
                    nc.tensor.matmul(
                        banks[q][m][:], whh[:, k, col], hprev[:, k, :],
                        start=(q == "hn" and k == 0), stop=(k == KT - 1),
                    )

            cur = new_banks()
            emit_x_mms(0, cur)
            hprev = None
            for t in range(KSIZE):
                if t > 0:
                    emit_h_mms(t, cur, hprev[:])
                if t < KSIZE - 1:
                    nxt = new_banks()
                    emit_x_mms(t + 1, nxt)
                else:
                    nxt = None

                # --- Elementwise chain for step t.
                r_t = work.tile([128, 2, NS], MM_DT, tag="rg")
                z_t = work.tile([128, 2, NS], MM_DT, tag="zg")
                tmp = work.tile([128, 2, NS], MM_DT, tag="tmp")
                pren = work.tile([128, 2, NS], MM_DT, tag="pren")
                n_t = work.tile([128, 2, NS], MM_DT, tag="n")
                d_t = work.tile([128, 2, NS], MM_DT, tag="d")
                e_t = work.tile([128, 2, NS], MM_DT, tag="e")
                hnew = hp.tile([128, 2, NS], MM_DT, tag="h")

                # Final step: run the whole chain in seq-halves so the last
                # output DMA starts sooner (nothing left to overlap with).
                spl = (
                    [(0, NS // 2), (NS // 2, NS)] if t == KSIZE - 1
                    else [(0, NS)]
                )
                for s0, s1 in spl:
                    # ScalarE queue: r0, r1, tanh0, z0, tanh1, z1 (z after
                    # the tanh it would otherwise delay; z feeds only e).
                    nc.scalar.activation(
                        r_t[:, 0, s0:s1], cur["r"][0][:, s0:s1], AF.Sigmoid,
                        bias=brz[:, 0:1],
                    )
                    nc.scalar.activation(
                        r_t[:, 1, s0:s1], cur["r"][1][:, s0:s1], AF.Sigmoid,
                        bias=brz[:, 1:2],
                    )

                    for m in range(2):
                        if t == 0:
                            nc.vector.tensor_scalar_mul(
                                tmp[:, m, s0:s1], r_t[:, m, s0:s1],
                                bhn[:, m : m + 1],
                            )
                        else:
                            nc.vector.scalar_tensor_tensor(
                                tmp[:, m, s0:s1], cur["hn"][m][:, s0:s1],
                                bhn[:, m : m + 1],
                                r_t[:, m, s0:s1], op0=OP.add, op1=OP.mult,
                            )
                        nc.vector.scalar_tensor_tensor(
                            pren[:, m, s0:s1], cur["in"][m][:, s0:s1],
                            bin_[:, m : m + 1],
                            tmp[:, m, s0:s1], op0=OP.add, op1=OP.add,
                        )
                        nc.scalar.activation(
                            n_t[:, m, s0:s1], pren[:, m, s0:s1], AF.Tanh
                        )
                        nc.scalar.activation(
                            z_t[:, m, s0:s1], cur["z"][m][:, s0:s1],
                            AF.Sigmoid, bias=brz[:, 2 + m : 3 + m],
                        )

                    # DVE tail per half: d = h_prev - n; e = z*d; h = n + e.
                    # (t=0: e = z*n; h = n - e.)
                    for m in range(2):
                        if t == 0:
                            nc.vector.tensor_tensor(
                                e_t[:, m, s0:s1], z_t[:, m, s0:s1],
                                n_t[:, m, s0:s1], op=OP.mult,
                            )
                            nc.vector.tensor_tensor(
                                hnew[:, m, s0:s1], n_t[:, m, s0:s1],
                                e_t[:, m, s0:s1], op=OP.subtract,
                            )
                        else:
                            nc.vector.tensor_tensor(
                                d_t[:, m, s0:s1], hprev[:, m, s0:s1],
                                n_t[:, m, s0:s1], op=OP.subtract,
                            )
                            nc.vector.tensor_tensor(
                                e_t[:, m, s0:s1], z_t[:, m, s0:s1],
                                d_t[:, m, s0:s1], op=OP.mult,
                            )
                            nc.vector.tensor_tensor(
                                hnew[:, m, s0:s1], n_t[:, m, s0:s1],
                                e_t[:, m, s0:s1], op=OP.add,
                            )
                        nc.gpsimd.dma_start(
                            out_d.ap()[t, m][:, s0:s1], hnew[:, m, s0:s1]
                        )

                hprev = hnew
                cur = nxt

    nc.compile()
    return nc


_NC_CACHE = None


def _get_nc():
    global _NC_CACHE
    if _NC_CACHE is None:
        _NC_CACHE = build_nc()
    return _NC_CACHE


def _prep_shared(W_ih, W_hh, b_ih, b_hh):
    wih_t = np.ascontiguousarray(
        W_ih.T.reshape(KT, 128, 3 * H).transpose(1, 0, 2)
    ).astype(NP_MM_DT)
    whh_t = np.ascontiguousarray(
        W_hh.T.reshape(KT, 128, 3 * H).transpose(1, 0, 2)
    ).astype(NP_MM_DT)
    wih_rz = np.ascontiguousarray(wih_t[:, :, 0:512])
    wih_n = np.ascontiguousarray(wih_t[:, :, 512:768])
    bsum = (b_ih + b_hh).astype(np.float32)
    brz = np.ascontiguousarray(bsum[: 2 * H].reshape(4, 128).T)
    bhn = np.ascontiguousarray(b_hh[2 * H :].reshape(2, 128).T)
    bin_ = np.ascontiguousarray(b_ih[2 * H :].reshape(2, 128).T)
    return wih_rz, wih_n, whh_t, brz, bhn, bin_


def _prep_core_inputs(x, shared, core):
    wih_rz, wih_n, whh_t, brz, bhn, bin_ = shared
    xc = x[core * ROWS_PER_CORE : (core + 1) * ROWS_PER_CORE]  # [4, S, I]
    xc = xc.reshape(NS, KSIZE, I)
    # xt[t, p, k, s] = xc[s, t, k*128+p]
    xt = np.ascontiguousarray(
        xc.reshape(NS, KSIZE, KT, 128).transpose(1, 3, 2, 0)
    ).astype(NP_MM_DT)
    return {
        "xt": xt,
        "wih_rz": wih_rz,
        "wih_n": wih_n,
        "whh_t": whh_t,
        "brz": brz,
        "bhn": bhn,
        "bin": bin_,
    }


def kernel(x, W_ih, W_hh, b_ih, b_hh, ksize):
    x = np.asarray(x, dtype=np.float32)
    W_ih = np.asarray(W_ih, dtype=np.float32)
    W_hh = np.asarray(W_hh, dtype=np.float32)
    b_ih = np.asarray(b_ih, dtype=np.float32)
    b_hh = np.asarray(b_hh, dtype=np.float32)
    assert int(ksize) == KSIZE and x.shape == (B, S, I)

    shared = _prep_shared(W_ih, W_hh, b_ih, b_hh)
    in_maps = [_prep_core_inputs(x, shared, c) for c in range(NCORES)]
    nc = _get_nc()
    res = run_bass_kernel_spmd(nc, in_maps, core_ids=list(range(NCORES)))

    out = np.empty((B, S, H), dtype=np.float32)
    for c in range(NCORES):
        oc = np.asarray(res.results[c]["out"]).astype(np.float32)  # [t,m,p,s]
        # h[seq=s, t, hdim=m*128+p]
        hc = oc.transpose(3, 0, 1, 2).reshape(NS, KSIZE, H)
        out[c * ROWS_PER_CORE : (c + 1) * ROWS_PER_CORE] = hc.reshape(
            ROWS_PER_CORE, S, H
        )
    return out


# revision 13
# speedup vs baseline: 1.0335x; 1.0161x over previous
"""Trainium2 Bass kernel for nn_LocalRNN (local GRU, chunked scan).

Problem: B=32, S=2048, I=H=256, ksize=16. Each ksize-chunk runs a GRU from
h0=0, so the 32*128=4096 chunks are independent length-16 GRU chains.

Sharding: data-parallel over chunks — core c gets batch rows [4c:4c+4],
i.e. 512 chains. Weights replicated.

Per-core layout ("transposed"): gate/hidden dim on partitions, chain (seq)
index on the free dim, all NS=512 chains in one matmul (N=512 = one PSUM
bank of fp32). Per step t, for each gate-half m (2 halves of 128):

  psum[gate_m, seqs] = W_ih_m @ x_t^T (+ W_hh_m @ h_{t-1}^T)   (PE, fp16)
  r = sigmoid(psum_r + b_r)                    (ScalarE bias port)
  z = sigmoid(psum_z + b_z)
  n = tanh((psum_in + b_in) + r*(psum_hn + b_hn))  (DVE stt ops + ScalarE)
  h = n + z*(h_prev - n)                       (DVE d/e/h chain)

The 8 PSUM banks hold r0,r1,z0,z1,in0,in1,hn0,hn1 single-buffered; step
t+1's x-side matmuls are emitted right after step t's h-side matmuls so the
PE pipelines across the elementwise chain (x-side needs no h). h-side
matmuls are ordered so k0-consumers lead (h half 0 lands ~1us before half 1).

Matmul operands and elementwise SBUF tensors are fp16 (values are O(1));
PSUM accumulation is fp32. Weight DMAs ride the Sync queue while x tiles
ride the GpSimd queue so the startup transfers overlap. Host pre-transposes
x / weights into DMA-friendly contiguous blocks and inverts the output
layout at the end.
"""

import sys

for _p in ("/opt/trn_rl_repo", "/root/.axon_site"):
    if _p not in sys.path:
        sys.path.insert(0, _p)

import ml_dtypes  # noqa: F401
import numpy as np

import concourse.bass as bass  # noqa: F401
import concourse.tile as tile
from concourse import bacc, mybir
from concourse.bass_utils import run_bass_kernel_spmd

# Problem constants (hardcoded per harness contract).
B, S, I, H = 32, 2048, 256, 256
KSIZE = 16
NCORES = 8
ROWS_PER_CORE = B // NCORES            # 4 batch rows per core
CHUNKS_PER_ROW = S // KSIZE            # 128
NS = ROWS_PER_CORE * CHUNKS_PER_ROW    # 512 chains per core
KT = 2                                 # contraction tiles (I/128 = H/128 = 2)

F32 = mybir.dt.float32
F16 = mybir.dt.float16
AF = mybir.ActivationFunctionType
OP = mybir.AluOpType

MM_DT = F16
NP_MM_DT = np.float16

# Gate column bases in the 3H weight layout.
GCOL = {"r": 0, "z": 256, "n": 512}


def build_nc():
    nc = bacc.Bacc("TRN2", target_bir_lowering=False, debug=False)

    # Inputs (host pre-transposed, contiguous per-DMA blocks).
    # xt[t, p, k, s] = x_shard[seq=s, t, i=k*128+p]
    xt_d = nc.dram_tensor("xt", [KSIZE, 128, KT, NS], MM_DT, kind="ExternalInput")
    # wih split so the first (x-side r/z) matmuls can start sooner.
    wih_rz_d = nc.dram_tensor("wih_rz", [128, KT, 512], MM_DT, kind="ExternalInput")
    wih_n_d = nc.dram_tensor("wih_n", [128, KT, 256], MM_DT, kind="ExternalInput")
    whh_d = nc.dram_tensor("whh_t", [128, KT, 3 * H], MM_DT, kind="ExternalInput")
    # brz[p, mi] = (b_ih+b_hh)[mi*128+p] for mi in r0,r1,z0,z1
    brz_d = nc.dram_tensor("brz", [128, 4], F32, kind="ExternalInput")
    # bhn[p, m] = b_hh[2H + m*128 + p]; bin[p, m] = b_ih[2H + m*128 + p]
    bhn_d = nc.dram_tensor("bhn", [128, 2], F32, kind="ExternalInput")
    bin_d = nc.dram_tensor("bin", [128, 2], F32, kind="ExternalInput")
    # out[t, m, p, s] = h_t[seq=s, hdim=m*128+p]
    out_d = nc.dram_tensor("out", [KSIZE, 2, 128, NS], MM_DT, kind="ExternalOutput")

    with tile.TileContext(nc) as tc:
        with (
            tc.tile_pool(name="consts", bufs=1) as consts,
            tc.tile_pool(name="xp", bufs=KSIZE) as xp,
            tc.tile_pool(name="ps", bufs=1, space="PSUM") as ps,
            tc.tile_pool(name="work", bufs=2) as work,
            tc.tile_pool(name="hp", bufs=3) as hp,
        ):
            # --- Input DMAs. Weights/biases dispatch from the Scalar queue,
            # x tiles from Sync, so the startup transfers overlap.
            wih = consts.tile([128, KT, 3 * H], MM_DT)
            nc.scalar.dma_start(wih[:, :, 0:512], wih_rz_d.ap())
            xs = []
            for t in range(KSIZE):
                xs.append(
                    xp.tile([128, KT, NS], MM_DT, tag="x", name=f"xs{t}")
                )
            nc.sync.dma_start(xs[0][:], xt_d.ap()[0])
            nc.scalar.dma_start(wih[:, :, 512:768], wih_n_d.ap())
            nc.sync.dma_start(xs[1][:], xt_d.ap()[1])
            brz = consts.tile([128, 4], F32)
            nc.scalar.dma_start(brz[:], brz_d.ap())
            bhn = consts.tile([128, 2], F32)
            nc.scalar.dma_start(bhn[:], bhn_d.ap())
            bin_ = consts.tile([128, 2], F32)
            nc.scalar.dma_start(bin_[:], bin_d.ap())
            whh = consts.tile([128, KT, 3 * H], MM_DT)
            nc.scalar.dma_start(whh[:], whh_d.ap())
            for t in range(2, KSIZE):
                nc.sync.dma_start(xs[t][:], xt_d.ap()[t])

            def new_banks():
                return {
                    q: [
                        ps.tile(
                            [128, NS], F32, tag=f"{q}{m}", name=f"bank_{q}{m}"
                        )
                        for m in range(2)
                    ]
                    for q in ("r", "z", "in", "hn")
                }

            def emit_x_mms(t, banks):
                """x-side matmuls for step t (emitted during step t-1).
                r/z first (their banks free earliest), in last."""
                stop_rz = t == 0  # no h-side at t=0: x k1 closes the group
                for q, m, stop in (
                    ("r", 0, stop_rz), ("r", 1, stop_rz),
                    ("in", 0, True), ("in", 1, True),
                    ("z", 0, stop_rz), ("z", 1, stop_rz),
                ):
                    col = slice(GCOL[q if q != "in" else "n"] + m * 128,
                                GCOL[q if q != "in" else "n"] + (m + 1) * 128)
                    for k in range(KT):
                        nc.tensor.matmul(
                            banks[q][m][:], wih[:, k, col], xs[t][:, k, :],
                            start=(k == 0), stop=(stop and k == KT - 1),
                        )

            def emit_h_mms(t, banks, hprev):
                """h-side matmuls for step t. k0-consumers first (h half 0 is
                ready ~1us before half 1); r gate leads the EW chain."""
                order = [
                    ("r", 0, 0), ("r", 1, 0), ("r", 0, 1), ("r", 1, 1),
                    ("hn", 0, 0), ("hn", 1, 0), ("hn", 0, 1), ("hn", 1, 1),
                    ("z", 0, 0), ("z", 0, 1), ("z", 1, 0), ("z", 1, 1),
                ]
                for q, m, k in order:
                    col = slice(GCOL[q if q != "hn" else "n"] + m * 128,
                                GCOL[q if q != "hn" else "n"] + (m + 1) * 128)
                    nc.tensor.matmul(
                        banks[q][m][:], whh[:, k, col], hprev[:, k, :],
                        start=(q == "hn" and k == 0), stop=(k == KT - 1),
                    )

            cur = new_banks()

            # --- PE warm-up: the HAM clock gate keeps the PE at 1.2 GHz
            # until it sees ~3.4us of sustained matmul activity. Run dummy
            # matmuls on a zeroed tile while the input DMAs are in flight so
            # the real stream starts at 2.4 GHz. The garbage written to bank
            # r0 is cleared by the first real (start=True) matmul.
            warm = consts.tile([128, 128], MM_DT)
            nc.vector.memset(warm[:], 0)
            for i in range(40):
                nc.tensor.matmul(
                    cur["r"][0][:, 0:64], warm[:], warm[:, 0:64],
                    start=(i == 0), stop=(i == 39), skip_group_check=True,
                )

            emit_x_mms(0, cur)
            hprev = None
            for t in range(KSIZE):
                if t > 0:
                    emit_h_mms(t, cur, hprev[:])
                if t < KSIZE - 1:
                    nxt = new_banks()
                    emit_x_mms(t + 1, nxt)
                else:
                    nxt = None

                # --- Elementwise chain for step t.
                r_t = work.tile([128, 2, NS], MM_DT, tag="rg")
                z_t = work.tile([128, 2, NS], MM_DT, tag="zg")
                tmp = work.tile([128, 2, NS], MM_DT, tag="tmp")
                pren = work.tile([128, 2, NS], MM_DT, tag="pren")
                n_t = work.tile([128, 2, NS], MM_DT, tag="n")
                d_t = work.tile([128, 2, NS], MM_DT, tag="d")
                e_t = work.tile([128, 2, NS], MM_DT, tag="e")
                hnew = hp.tile([128, 2, NS], MM_DT, tag="h")

                # Final step: run the whole chain in seq-halves so the last
                # output DMA starts sooner (nothing left to overlap with).
                spl = (
                    [(0, NS // 2), (NS // 2, NS)] if t == KSIZE - 1
                    else [(0, NS)]
                )
                for s0, s1 in spl:
                    # ScalarE queue: r0, r1, tanh0, z0, tanh1, z1 (z after
                    # the tanh it would otherwise delay; z feeds only e).
                    nc.scalar.activation(
                        r_t[:, 0, s0:s1], cur["r"][0][:, s0:s1], AF.Sigmoid,
                        bias=brz[:, 0:1],
                    )
                    nc.scalar.activation(
                        r_t[:, 1, s0:s1], cur["r"][1][:, s0:s1], AF.Sigmoid,
                        bias=brz[:, 1:2],
                    )

                    for m in range(2):
                        if t == 0:
                            nc.vector.tensor_scalar_mul(
                                tmp[:, m, s0:s1], r_t[:, m, s0:s1],
                                bhn[:, m : m + 1],
                            )
                        else:
                            nc.vector.scalar_tensor_tensor(
                                tmp[:, m, s0:s1], cur["hn"][m][:, s0:s1],
                                bhn[:, m : m + 1],
                                r_t[:, m, s0:s1], op0=OP.add, op1=OP.mult,
                            )
                        nc.vector.scalar_tensor_tensor(
                            pren[:, m, s0:s1], cur["in"][m][:, s0:s1],
                            bin_[:, m : m + 1],
                            tmp[:, m, s0:s1], op0=OP.add, op1=OP.add,
                        )
                        nc.scalar.activation(
                            n_t[:, m, s0:s1], pren[:, m, s0:s1], AF.Tanh
                        )
                        nc.scalar.activation(
                            z_t[:, m, s0:s1], cur["z"][m][:, s0:s1],
                            AF.Sigmoid, bias=brz[:, 2 + m : 3 + m],
                        )

                    # DVE tail per half: d = h_prev - n; e = z*d; h = n + e.
                    # (t=0: e = z*n; h = n - e.)
                    for m in range(2):
                        if t == 0:
                            nc.vector.tensor_tensor(
                                e_t[:, m, s0:s1], z_t[:, m, s0:s1],
                                n_t[:, m, s0:s1], op=OP.mult,
                            )
                            nc.vector.tensor_tensor(
                                hnew[:, m, s0:s1], n_t[:, m, s0:s1],
                                e_t[:, m, s0:s1], op=OP.subtract,
                            )
                        else:
                            nc.vector.tensor_tensor(
                                d_t[:, m, s0:s1], hprev[:, m, s0:s1],
                                n_t[:, m, s0:s1], op=OP.subtract,
                            )
                            nc.vector.tensor_tensor(
                                e_t[:, m, s0:s1], z_t[:, m, s0:s1],
                                d_t[:, m, s0:s1], op=OP.mult,
                            )
                            nc.vector.tensor_tensor(
                                hnew[:, m, s0:s1], n_t[:, m, s0:s1],
                                e_t[:, m, s0:s1], op=OP.add,
                            )
                        nc.gpsimd.dma_start(
                            out_d.ap()[t, m][:, s0:s1], hnew[:, m, s0:s1]
                        )

                hprev = hnew
                cur = nxt

    nc.compile()
    return nc


_NC_CACHE = None


def _get_nc():
    global _NC_CACHE
    if _NC_CACHE is None:
        _NC_CACHE = build_nc()
    return _NC_CACHE


def _prep_shared(W_ih, W_hh, b_ih, b_hh):
    wih_t = np.ascontiguousarray(
        W_ih.T.reshape(KT, 128, 3 * H).transpose(1, 0, 2)
    ).astype(NP_MM_DT)
    whh_t = np.ascontiguousarray(
        W_hh.T.reshape(KT, 128, 3 * H).transpose(1, 0, 2)
    ).astype(NP_MM_DT)
    wih_rz = np.ascontiguousarray(wih_t[:, :, 0:512])
    wih_n = np.ascontiguousarray(wih_t[:, :, 512:768])
    bsum = (b_ih + b_hh).astype(np.float32)
    brz = np.ascontiguousarray(bsum[: 2 * H].reshape(4, 128).T)
    bhn = np.ascontiguousarray(b_hh[2 * H :].reshape(2, 128).T)
    bin_ = np.ascontiguousarray(b_ih[2 * H :].reshape(2, 128).T)
    return wih_rz, wih_n, whh_t, brz, bhn, bin_


def _prep_core_inputs(x, shared, core):
    wih_rz, wih_n, whh_t, brz, bhn, bin_ = shared
    xc = x[core * ROWS_PER_CORE : (core + 1) * ROWS_PER_CORE]  # [4, S, I]
    xc = xc.reshape(NS, KSIZE, I)
    # xt[t, p, k, s] = xc[s, t, k*128+p]
    xt = np.ascontiguousarray(
        xc.reshape(NS, KSIZE, KT, 128).transpose(1, 3, 2, 0)
    ).astype(NP_MM_DT)
    return {
        "xt": xt,
        "wih_rz": wih_rz,
        "wih_n": wih_n,
        "whh_t": whh_t,
        "brz": brz,
        "bhn": bhn,
        "bin": bin_,
    }


def kernel(x, W_ih, W_hh, b_ih, b_hh, ksize):
    x = np.asarray(x, dtype=np.float32)
    W_ih = np.asarray(W_ih, dtype=np.float32)
    W_hh = np.asarray(W_hh, dtype=np.float32)
    b_ih = np.asarray(b_ih, dtype=np.float32)
    b_hh = np.asarray(b_hh, dtype=np.float32)
    assert int(ksize) == KSIZE and x.shape == (B, S, I)

    shared = _prep_shared(W_ih, W_hh, b_ih, b_hh)
    in_maps = [_prep_core_inputs(x, shared, c) for c in range(NCORES)]
    nc = _get_nc()
    res = run_bass_kernel_spmd(nc, in_maps, core_ids=list(range(NCORES)))

    out = np.empty((B, S, H), dtype=np.float32)
    for c in range(NCORES):
        oc = np.asarray(res.results[c]["out"]).astype(np.float32)  # [t,m,p,s]
        # h[seq=s, t, hdim=m*128+p]
        hc = oc.transpose(3, 0, 1, 2).reshape(NS, KSIZE, H)
        out[c * ROWS_PER_CORE : (c + 1) * ROWS_PER_CORE] = hc.reshape(
            ROWS_PER_CORE, S, H
        )
    return out
